# revision 27
# baseline (speedup 1.0000x reference)
"""Block-diagonal masked dense + BatchNorm(train) + ReLU on 8 TRN2 NeuronCores.

Math: out = x @ (W * blockdiag_mask) + bias; BN over batch; relu.
The mask keeps 64 diagonal blocks of shape [64 in, 64 out]. Group g only
couples x[:, 64g:64g+64] to out[:, 64g:64g+64].

Sharding: groups are split across cores (8 groups per core). Each core owns a
disjoint 512-column slice of both input and output features, so the matmul and
the per-feature batch statistics are fully core-local (no collectives).

Per-core device program (all shapes hardcoded):
  inputs:  xT [512, 4096] bf16 (x slice transposed on host), wd [128, 512]
           bf16 (partition-major; per 128-row chunk a 2x2 block-diagonal of
           two 64x64 group blocks), gb [128, 8] f32 (gamma|beta packed)
  output:  yT [512, 4096] bf16 (host transposes back, fp32-casts)

Pipeline:
  - input streams chunk-major on both HWDGE queues (sync + scalar): chunk-0
    weights alone first (unblocks LDWEIGHTS), chunk 0 as 128 KB tiles
    (earliest bn_stats start; a dependent op waits for a WHOLE transfer and
    each DMA costs ~0.65 us issue + ~2.5 us to usable), chunk 1 as 256 KB
    quarters, chunks 2-3 as quarters too (finer completion semaphores
    keep the bn_stats chain fed). scalar's issues land before ACT computes.
  - pass 1 per chunk: 8 matmuls K=128/N=512 -> PSUM, bn_stats (DVE) right
    behind each; coef chain = bn_aggr + reciprocal + A-mult on DVE with only
    the Sqrt and B algebra on ACT.
  - pass 2 per chunk: recompute the matmul (x stays SBUF-resident) and fold
    BN+relu into one ScalarE activation per 1024-col mega, PSUM -> SBUF
    bf16, store immediately (256 KB per store) on the gpsimd SWDGE queue so
    the output overlaps the remaining input.
  - last chunk: the final mega's relu reads the still-resident pass-1
    PSUM tiles directly on DVE (nothing reuses psum1 after them, so the
    recompute matmuls and the psum2 wait disappear), emitted ahead of the
    PSUM-blocked mega-1 ops in DVE's in-order queue; relu megas split
    ACT/DVE and stores split sync/gpsimd to shorten the serial drain.

The coef(c+1) emission is hoisted before chunk c's last relu mega so its
ACT ops (Sqrt, B algebra) sit ahead of that mega in ACT's in-order queue;
without this the DVE chain stalls 1-3.4 us per chunk waiting for Sqrt
(measured ~1 us end-to-end win).

Timing notes (measured): exec ~51.0 us mean, best rep 49.3 us, against
a 23.8 us DMA roofline. The
gap is toolchain-fixed serialization, profiled and attacked without success
from several directions: every matmul re-loads its stationary (walrus emits
LDWEIGHTS per matmul; --enable-ldw-opt crashes codegen; a stripped weights
operand is rejected by the verifier), so free-running matmuls cost the
isolated ~379 ns rather than the 216 ns pipelined rate; the in-order PE
queue makes the stats-paced pass-1 waits additive with pass-2 work; the
DVE bn_stats chain (32 x ~680 ns, no fast perf mode exists for bn_stats)
plus ~60 semaphore-wait ops per engine (~8 us) pace the rest; and DMA
transfers only become usable ~2.5-4 us after issue. Schedule variants
(finer/coarser DMA, 3-queue input, grain-4 matmul grouping, all-DVE coef
with Newton rsqrt, psum pool rebalances) all measured equal or worse.
Note: invocation-level device variance is +-4-8 us on this box; compare
schedules only with interleaved multi-rep runs.

Accuracy: ~3e-3 rel L2 vs the fp32 reference (bf16 I/O rounding; BN math and
PSUM accumulation in fp32). bias never reaches the device: BN's mean
subtraction absorbs it exactly and variance is shift-invariant.
"""

import numpy as np

import concourse.bass as bass
import concourse.tile as tile
from concourse import mybir
from concourse.bass_utils import run_bass_kernel_spmd

F32 = mybir.dt.float32
BF16 = mybir.dt.bfloat16

NCORES = 8
BATCH = 4096
DIM = 4096
DCORE = DIM // NCORES          # 512 features per core
CHUNKS = DCORE // 128          # 4 partition chunks (2 groups each)
BTILE = 512                    # bn_stats tile (FMAX) / one PSUM bank
BTILES = BATCH // BTILE        # 8
MEGA = 1024                    # relu/store granularity (2 PSUM banks)
MEGAS = BATCH // MEGA          # 4
EPS = 1e-3

_MAX_WAITS = 1


def _split_multi_waits(nc: bass.Bass, max_waits: int = _MAX_WAITS) -> None:
    # The walrus build in this container rejects instructions carrying more
    # than one sync-wait command (any engine, any opcode). Hoist extra waits
    # onto same-engine NOPs inserted immediately before the instruction —
    # identical semantics, since the engine blocks on each wait in order.
    # Snapshot every block BEFORE creating any nop: the engine builders append
    # new instructions to the current (last) block as a side effect, and the
    # final wholesale reassignment below discards those spurious appends.
    snapshots = [
        (bb, list(bb.instructions)) for f in nc.m.functions for bb in f.blocks
    ]
    rebuilt = []
    for bb, insts in snapshots:
        new = []
        for ins in insts:
            si = getattr(ins, "sync_info", None)
            waits = list(si.on_wait) if si is not None and si.on_wait else []
            if len(waits) > max_waits:
                head = waits[:-max_waits]
                for i in range(0, len(head), max_waits):
                    nop = nc.engines[ins.engine].nop().ins
                    nop.sync_info = mybir.SyncInfo(
                        on_wait=head[i : i + max_waits], on_update=[]
                    )
                    new.append(nop)
                ins.sync_info = mybir.SyncInfo(
                    on_wait=waits[-max_waits:],
                    on_update=list(si.on_update or []),
                )
            new.append(ins)
        rebuilt.append((bb, new))
    for bb, new in rebuilt:
        bb.instructions = new


def _build_nc() -> bass.Bass:
    nc = bass.Bass()
    xT = nc.dram_tensor("xT", [DCORE, BATCH], BF16, kind="ExternalInput")
    # Partition-major [128, c*m]: one contiguous 1 KB descriptor per
    # partition (the [DCORE, 128] layout shattered into 512 x 256 B
    # descriptors and took ~3 us to land, stalling the first matmul).
    wd = nc.dram_tensor("wd", [128, CHUNKS * 128], BF16, kind="ExternalInput")
    # gamma/beta packed into one partition-major tensor: a single DMA.
    gb = nc.dram_tensor("gb", [128, 2 * CHUNKS], F32, kind="ExternalInput")
    yT = nc.dram_tensor("yT", [DCORE, BATCH], BF16, kind="ExternalOutput")

    with tile.TileContext(nc) as tc:
        with (
            tc.tile_pool(name="singles", bufs=1) as singles,
            tc.tile_pool(name="stats", bufs=1) as statp,
            tc.tile_pool(name="psum1", bufs=2, space="PSUM") as psum1,
            tc.tile_pool(name="psum2", bufs=3, space="PSUM") as psum2,
        ):
            xsb = singles.tile([128, CHUNKS, BATCH], BF16)
            xTv = xT.rearrange("(c p) b -> p c b", p=128)
            wsb = singles.tile([128, CHUNKS, 128], BF16)
            gbs = singles.tile([128, 2, CHUNKS], F32)
            gsb = gbs[:, 0, :]
            bsb = gbs[:, 1, :]
            zsb = singles.tile([128, CHUNKS, BATCH], BF16)
            yTv = yT.rearrange("(c p) b -> p c b", p=128)

            # Input plan. Every DMA costs ~0.65 us of issue time on its
            # engine plus ~1.5 us issue->data and ~1 us data->semaphore
            # latency, and a dependent op waits for the WHOLE transfer. So:
            # chunk-0 weights alone first (32 KB, unblocks LDWEIGHTS), then
            # chunk 0 in 128 KB tiles alternating queues (earliest possible
            # bn_stats start), coarser pieces for later chunks. scalar (=ACT)
            # issues all land before ACT has compute to do.
            wdv = wd.rearrange("p (c m) -> p c m", c=CHUNKS)
            # w0 heads the scalar queue while t0 heads sync: the first
            # matmul's two dependencies (weights + data) land in parallel
            # instead of back-to-back on one queue.
            nc.scalar.dma_start(wsb[:, 0:1, :], wdv[:, 0:1, :])
            T = BTILE
            for t in range(4):
                eng = nc.sync if t % 2 == 0 else nc.scalar
                eng.dma_start(
                    xsb[:, 0, t * T : (t + 1) * T], xTv[:, 0, t * T : (t + 1) * T]
                )
            nc.scalar.dma_start(gbs[:], gb.rearrange("p (g c) -> p g c", g=2))
            nc.sync.dma_start(wsb[:, 1:, :], wdv[:, 1:, :])
            for t in range(4, 8):
                eng = nc.sync if t % 2 == 0 else nc.scalar
                eng.dma_start(
                    xsb[:, 0, t * T : (t + 1) * T], xTv[:, 0, t * T : (t + 1) * T]
                )
            Q = 1024
            for p in range(4):
                eng = nc.sync if p % 2 == 0 else nc.scalar
                eng.dma_start(
                    xsb[:, 1, p * Q : (p + 1) * Q], xTv[:, 1, p * Q : (p + 1) * Q]
                )
            for c in range(2, CHUNKS):
                for p in range(4):
                    eng = nc.sync if p % 2 == 0 else nc.scalar
                    eng.dma_start(
                        xsb[:, c, p * Q : (p + 1) * Q],
                        xTv[:, c, p * Q : (p + 1) * Q],
                    )

            epsb = singles.tile([128, 1], F32)
            nc.vector.memset(epsb[:], EPS)

            stats = statp.tile([128, CHUNKS, BTILES, 6], F32)
            mv = statp.tile([128, CHUNKS, 2], F32)
            coefA = statp.tile([128, CHUNKS], F32)
            coefB = statp.tile([128, CHUNKS], F32)
            tmp = statp.tile([128, CHUNKS], F32)

            def one_matmul(ps, os, c: int, t: int):
                # K=128 against a 2x2 block-diagonal stationary (two 64x64
                # group blocks; zeros kill the cross terms).
                nc.tensor.matmul(
                    ps[:, os],
                    lhsT=wsb[:, c, :],
                    rhs=xsb[:, c, bass.ds(t * BTILE, BTILE)],
                    start=True, stop=True,
                )

            c3_res = {}

            def p1_tile(c: int, t: int):
                ps = psum1.tile([128, BTILE], F32, tag="ps1")
                one_matmul(ps, slice(None), c, t)
                nc.vector.bn_stats(stats[:, c, t, :], ps[:, :])
                if c == CHUNKS - 1 and t >= BTILES - 2:
                    # Nothing allocates from psum1 after the last chunk's
                    # last two tiles: they stay resident so mega 3's relu
                    # can read them directly (no recompute matmuls).
                    c3_res[t] = ps

            def coef(c: int):
                # The DVE bn_stats chain is the kernel's critical path, so
                # it gets few inserts per chunk: bn_aggr + reciprocal +
                # A-multiply (all DVE-only ops); Sqrt and the B algebra ride
                # ACT in parallel with the next chunk's stats.
                nc.vector.bn_aggr(mv[:, c, :], stats[:, c, :, :])
                nc.scalar.activation(
                    tmp[:, c : c + 1], mv[:, c, 1:2],
                    mybir.ActivationFunctionType.Sqrt,
                    bias=epsb[:], scale=1.0,
                )
                nc.vector.reciprocal(tmp[:, c : c + 1], tmp[:, c : c + 1])
                # A = gamma * rsqrt(var+eps); B = beta - mean * A. All on
                # DVE: ACT ops here would queue behind in-flight relu megas
                # (measured +3.3 us of coef latency on the last chunk).
                nc.vector.tensor_tensor(
                    coefA[:, c : c + 1], gsb[:, c : c + 1],
                    tmp[:, c : c + 1], mybir.AluOpType.mult,
                )
                nc.vector.tensor_tensor(
                    tmp[:, c : c + 1], mv[:, c, 0:1],
                    coefA[:, c : c + 1], mybir.AluOpType.mult,
                )
                nc.vector.scalar_tensor_tensor(
                    coefB[:, c : c + 1], tmp[:, c : c + 1], -1.0,
                    bsb[:, c : c + 1],
                    op0=mybir.AluOpType.mult, op1=mybir.AluOpType.add,
                )

            def m3_direct():
                # relu(A*y+B) for the last chunk's last mega straight from
                # the still-resident pass-1 PSUM tiles, on DVE, ready the
                # moment coef(c3) lands.
                c = CHUNKS - 1
                for q in range(2):
                    t = BTILES - 2 + q
                    sl = bass.ds(t * BTILE, BTILE)
                    nc.vector.tensor_scalar(
                        zsb[:, c, sl], c3_res[t][:],
                        coefA[:, c : c + 1], coefB[:, c : c + 1],
                        mybir.AluOpType.mult, mybir.AluOpType.add,
                    )
                    nc.vector.tensor_scalar(
                        zsb[:, c, sl], zsb[:, c, sl],
                        0.0, None, mybir.AluOpType.max,
                    )
                msl = bass.ds((MEGAS - 1) * MEGA, MEGA)
                nc.gpsimd.dma_start(yTv[:, c, msl], zsb[:, c, msl])

            def p2_mega(c: int, m: int):
                # Recompute the matmul (x stays SBUF-resident) and fold
                # BN+relu into one pass, PSUM -> SBUF bf16 -> DRAM. The last
                # chunk's megas alternate ACT/DVE and sync/gpsimd stores to
                # shorten the final serial drain.
                ps = psum2.tile([128, MEGA], F32, tag="ps2")
                for q in range(MEGA // BTILE):
                    one_matmul(
                        ps, bass.ds(q * BTILE, BTILE), c,
                        m * (MEGA // BTILE) + q,
                    )
                msl = bass.ds(m * MEGA, MEGA)
                last = c == CHUNKS - 1
                if last and m % 2 == 1:
                    # z = relu(A*y + B) on DVE: affine (PSUM src, 1x) then
                    # max(0) at 4x. Frees ACT for the other megas.
                    nc.vector.tensor_scalar(
                        zsb[:, c, msl], ps[:],
                        coefA[:, c : c + 1], coefB[:, c : c + 1],
                        mybir.AluOpType.mult, mybir.AluOpType.add,
                    )
                    nc.vector.tensor_scalar(
                        zsb[:, c, msl], zsb[:, c, msl],
                        0.0, None, mybir.AluOpType.max,
                    )
                else:
                    nc.scalar.activation(
                        zsb[:, c, msl], ps[:],
                        mybir.ActivationFunctionType.Relu,
                        bias=coefB[:, c : c + 1], scale=coefA[:, c : c + 1],
                    )
                if last and m == 2:
                    # This mega's relu is the last compute op; its store
                    # gates the end of the kernel. Halve it across both
                    # free queues so the drains run in parallel.
                    h0 = bass.ds(m * MEGA, MEGA // 2)
                    h1 = bass.ds(m * MEGA + MEGA // 2, MEGA // 2)
                    nc.sync.dma_start(yTv[:, c, h0], zsb[:, c, h0])
                    nc.gpsimd.dma_start(yTv[:, c, h1], zsb[:, c, h1])
                else:
                    eng = nc.sync if (last and m % 2 == 0) else nc.gpsimd
                    eng.dma_start(yTv[:, c, msl], zsb[:, c, msl])

            for t in range(BTILES):
                p1_tile(0, t)
            coef(0)
            for c in range(CHUNKS):
                for m in range(MEGAS):
                    # p2 megas sit AHEAD of the stats-paced p1 waits in the
                    # PE's in-order FIFO, so the relu feed is not delayed by
                    # them (the p1 matmuls stall on psum1-frees regardless
                    # of queue position). Exception: the last mega follows
                    # the hoisted coef(c+1) so coef's ACT Sqrt stays ahead
                    # of it in ACT's queue.
                    if c == CHUNKS - 1:
                        if m == 1:
                            # DVE's in-order queue takes m3-direct BEFORE
                            # m1: m3's inputs are already resident in PSUM
                            # at coef time, while m1 waits on its recompute
                            # matmuls (wrong order measured ~2.5 us).
                            m3_direct()
                        elif m == MEGAS - 1:
                            p2_mega(c, 1)
                        else:
                            p2_mega(c, m)
                    elif m < MEGAS - 1:
                        p2_mega(c, m)
                    if c + 1 < CHUNKS:
                        p1_tile(c + 1, 2 * m)
                        p1_tile(c + 1, 2 * m + 1)
                        if m == MEGAS - 1:
                            coef(c + 1)
                            p2_mega(c, m)
    _split_multi_waits(nc)
    return nc


_NC_CACHE: bass.Bass | None = None


def _get_nc() -> bass.Bass:
    global _NC_CACHE
    if _NC_CACHE is None:
        _NC_CACHE = _build_nc()
    return _NC_CACHE


from ml_dtypes import bfloat16 as _bf16


def _make_in_maps(x, weight, gamma, beta):
    in_maps = []
    for c in range(NCORES):
        sl = slice(c * DCORE, (c + 1) * DCORE)
        xT = np.ascontiguousarray(x[:, sl].T).astype(_bf16)
        # Per 128-row chunk: [[w_{2c}, 0], [0, w_{2c+1}]] block-diagonal.
        wdc = np.zeros((DCORE, 128), np.float32)
        for g in range(DCORE // 64):
            r = slice(c * DCORE + g * 64, c * DCORE + (g + 1) * 64)
            col = (g % 2) * 64
            wdc[g * 64 : (g + 1) * 64, col : col + 64] = weight[r, r]
        # Partition-major: wd2[p, 128c+m] = wdc[128c+p, m]; 1 KB contiguous
        # per partition so the weight DMA is one descriptor per partition.
        wd2 = np.ascontiguousarray(
            wdc.reshape(CHUNKS, 128, 128).transpose(1, 0, 2).reshape(128, -1)
        )
        g2 = np.ascontiguousarray(gamma[sl].reshape(CHUNKS, 128).T)
        b2 = np.ascontiguousarray(beta[sl].reshape(CHUNKS, 128).T)
        in_maps.append(
            {
                "xT": xT,
                "wd": wd2.astype(_bf16),
                "gb": np.concatenate([g2, b2], axis=1).astype(np.float32),
            }
        )
    return in_maps


def kernel(x, weight, bias, gamma, beta, **_run_kwargs) -> np.ndarray:
    x = np.asarray(x, np.float32)
    weight = np.asarray(weight, np.float32)
    gamma = np.asarray(gamma, np.float32)
    beta = np.asarray(beta, np.float32)
    # bias is algebraically irrelevant: BN subtracts the batch mean, which
    # absorbs any constant per-feature shift, and variance is shift-invariant.

    nc = _get_nc()
    res = run_bass_kernel_spmd(
        nc, _make_in_maps(x, weight, gamma, beta),
        core_ids=list(range(NCORES)), **_run_kwargs,
    )
    out = np.empty((BATCH, DIM), np.float32)
    for c, r in enumerate(res.results):
        out[:, c * DCORE : (c + 1) * DCORE] = r["yT"].T.astype(np.float32)
    kernel.last_results = res
    return out
